# revision 14
# baseline (speedup 1.0000x reference)
"""Sparse (sigmoid) attention block on 8 TRN2 NeuronCores — v2.

Sharding: core c = (batch b=c//2, head-half hh=c%2).  Projection + attention
are head-split (6 of 12 heads per core, full 2048-row sequence).  LayerNorm
needs full-hidden row statistics, so per 512-row block the two cores of a
pair AllReduce their partial (sum, sum-of-squares) stats — 4 KB each — and
then normalize/gate only their own hidden half; the output projection is
contraction-split and a bf16 ReduceScatter(add) hands each core its 384
output columns.  This kills the v1 AllGather design's duplicated U
projection / stats / gate work.

Key layout choices vs the v1 kernel:
  * Q^T/K^T are produced DIRECTLY by W-stationary matmuls (out partitions =
    head-pair dims), so the 96 PE transposes and all ScalarE PSUM copies are
    gone.  RoPE is applied in transposed layout: the partition-rotation
    (rotate-half) runs as 4 quarter-multiplies on the otherwise-idle GpSimd
    engine, plus 2 DVE ops.
  * Causal windowing: for a diagonal key-chunk j of a 512-query block only
    the valid query window [128j:512] is computed (scores matmuls, sigmoid,
    and A@V all shrink); the triangular mask multiply is only [128,128].
  * LN statistics come from ones-matmuls on the PE; rstd is computed on the
    DVE with a bit-trick rsqrt + 2 Newton steps so ScalarE never has to
    switch activation table sets (sigmoid only).
All heavy compute bf16 with f32 PSUM accumulation.
"""

import numpy as np
import ml_dtypes

import concourse.bass as bass
import concourse.bacc as bacc
import concourse.mybir as mybir
import concourse.tile as tile
from concourse import bass_utils

BF16 = mybir.dt.bfloat16
F32 = mybir.dt.float32
I32 = mybir.dt.int32
AF = mybir.ActivationFunctionType
ALU = mybir.AluOpType

S = 2048          # sequence length
HID = 768         # hidden
NHEADS = 12       # total heads
D = 64            # head dim
NH = 6            # heads per core
NPAIR = 3         # head pairs per core
QB = 512          # query block
CT = 6            # hidden c-tiles of 128
LN_EPS = 1e-8
N_CORES = 8
RSQRT_MAGIC = 0x5F3759DF


def _rope_tables():
    inv_freq = 1.0 / (10000.0 ** (np.arange(0, D, 2, dtype=np.float64) / D))
    t = np.arange(S, dtype=np.float64)
    freqs = np.outer(t, inv_freq)                      # [S, 32]
    emb = np.concatenate([freqs, freqs], axis=-1)      # [S, 64]
    return np.cos(emb).astype(np.float32), np.sin(emb).astype(np.float32)


def build_nc(ndev, pairs):
    """Emit the per-core Bass/Tile graph (identical for every core)."""
    nc = bacc.Bacc("TRN2", target_bir_lowering=False, debug=False,
                   num_devices=ndev)

    def din(name, shape, dt):
        return nc.dram_tensor(name, shape, dt, kind="ExternalInput").ap()

    xT = din("xT", [HID, S], BF16)                 # full rows
    w_qkv = din("w_qkv", [HID, 3 * NH * D], BF16)  # Q|K|V col blocks
    w_u = din("w_u", [HID, NH * D], BF16)          # its u-col half
    w_out = din("w_out", [NH * D, HID], BF16)      # its rows, gamma-folded
    cosT = din("cosT", [128, S], BF16)             # [2x64 dims, seq]
    sinT = din("sinT", [128, S], BF16)             # sign-folded
    maskb = din("maskb", [128, 128], BF16)         # c >= i
    ones_k = din("ones_k", [128, 1], BF16)
    residT = din("residT", [NH * D, S], F32)       # x^T col-half + b_out
    out = nc.dram_tensor("out", [NH * D, S], F32, kind="ExternalOutput").ap()

    xT_r = xT.rearrange("(k p) s -> p k s", p=128)         # [128, 6, S]
    wqkv_r = w_qkv.rearrange("(k p) c -> p k c", p=128)    # [128, 6, 1152]
    wu_r = w_u.rearrange("(k p) c -> p k c", p=128)        # [128, 6, 384]
    wout_r = w_out.rearrange("(k p) c -> p k c", p=128)    # [128, 3, 768]
    residT_r = residT.rearrange("(c p) s -> p c s", p=128)  # [128, 3, S]
    out_r = out.rearrange("(c p) s -> p c s", p=128)

    with tile.TileContext(nc) as tc:
        _emit(nc, tc, pairs, xT_r, wqkv_r, wu_r, wout_r, cosT, sinT,
              maskb, ones_k, residT_r, out_r)
    nc.compile()
    return nc


def _emit(nc, tc, pairs, xT_r, wqkv_r, wu_r, wout_r, cosT, sinT,
          maskb, ones_k, residT_r, out_r):
    from contextlib import ExitStack
    es = ExitStack()
    with es:
        # ---- resident SBUF tensors -----------------------------------
        res = es.enter_context(tc.tile_pool(name="resident", bufs=1))
        xT_sb = res.tile([128, 6, S], BF16, tag="xT")
        wqkv_sb = res.tile([128, 6, 3 * NH * D], BF16, tag="wqkv")
        wu_sb = res.tile([128, 6, NH * D], BF16, tag="wu")
        wout_sb = res.tile([128, NPAIR, HID], BF16, tag="wout")
        cosT_sb = res.tile([128, S], BF16, tag="cosT")
        sinT_sb = res.tile([128, S], BF16, tag="sinT")
        maskb_sb = res.tile([128, 128], BF16, tag="maskb")
        ones_sb = res.tile([128, 1], BF16, tag="ones")
        qt_sb = res.tile([128, NPAIR, S], BF16, tag="qt")   # Q^T slab
        kt_sb = res.tile([128, NPAIR, S], BF16, tag="kt")   # K^T slab
        v_sb = res.tile([128, 16, NH, D], BF16, tag="v")    # V row slab
        ut_sb = res.tile([128, NPAIR, S], BF16, tag="ut")   # silu(U)^T half
        ao_sb = res.tile([128, NPAIR, S], BF16, tag="ao")   # attn out^T

        # loads: QKV-proj critical first
        for k in range(6):
            nc.sync.dma_start(out=xT_sb[:, k, :], in_=xT_r[:, k, :])
            nc.sync.dma_start(out=wqkv_sb[:, k, :], in_=wqkv_r[:, k, :])
        nc.sync.dma_start(out=cosT_sb[:], in_=cosT[:])
        nc.sync.dma_start(out=sinT_sb[:], in_=sinT[:])
        nc.sync.dma_start(out=maskb_sb[:], in_=maskb[:])
        nc.sync.dma_start(out=ones_sb[:], in_=ones_k[:])
        for k in range(6):
            nc.sync.dma_start(out=wu_sb[:, k, :], in_=wu_r[:, k, :])
        for p in range(NPAIR):
            nc.sync.dma_start(out=wout_sb[:, p, :], in_=wout_r[:, p, :])

        # ---- pools ---------------------------------------------------
        dram = es.enter_context(tc.tile_pool(name="ccdram", bufs=4,
                                             space="DRAM"))
        pjp = es.enter_context(tc.tile_pool(name="pj", bufs=2, space="PSUM"))
        scp = es.enter_context(tc.tile_pool(name="sc", bufs=2, space="PSUM"))
        avp = es.enter_context(tc.tile_pool(name="av", bufs=2, space="PSUM"))
        atp = es.enter_context(tc.tile_pool(name="at", bufs=9))
        wrk = es.enter_context(tc.tile_pool(name="wrk", bufs=2))
        ep = es.enter_context(tc.tile_pool(name="ep", bufs=2))
        gp = es.enter_context(tc.tile_pool(name="gp", bufs=1))

        # ---------- emit helpers --------------------------------------
        def rope_tile(dst_slab, p, sb, pq):
            """RoPE in transposed layout: dst = pq*cos + rot32(pq)*sinN.

            pq: [128, 512] f32 PSUM (2 heads x 64 dims).  rot32 swaps
            partition halves within each 64-dim head: those 4 quarter
            multiplies run on GpSimd (cross-partition-base), the rest DVE.
            """
            s0, s1 = sb * QB, (sb + 1) * QB
            t1 = wrk.tile([128, QB], BF16, tag="ropet1")
            nc.vector.tensor_mul(t1[:], pq[:], cosT_sb[:, s0:s1])
            t2 = wrk.tile([128, QB], BF16, tag="ropet2")
            for (a, b) in ((0, 32), (32, 0), (64, 96), (96, 64)):
                nc.vector.tensor_mul(t2[a:a + 32, :], pq[b:b + 32, :],
                                     sinT_sb[a:a + 32, s0:s1])
            nc.vector.tensor_add(dst_slab[:, p, s0:s1], t1[:], t2[:])

        def proj_qkt(sb):
            """Q^T and K^T tiles for 512-row block sb (W-stationary)."""
            s0, s1 = sb * QB, (sb + 1) * QB
            # w_qkv cols: Q at [0:384], K at [384:768]
            for off, dst in ((0, qt_sb), (NH * D, kt_sb)):
                for p in range(NPAIR):
                    pq = pjp.tile([128, QB], F32, tag="pj", name=f"pq{sb}{p}")
                    for k in range(6):
                        nc.tensor.matmul(
                            pq[:],
                            wqkv_sb[:, k, off + p * 128: off + (p + 1) * 128],
                            xT_sb[:, k, s0:s1],
                            start=(k == 0), stop=(k == 5))
                    rope_tile(dst, p, sb, pq)

        def proj_v(sb):
            """V row-layout tiles for block sb (x-stationary)."""
            for rt4 in range(4):
                rt = sb * 4 + rt4
                pv = pjp.tile([128, NH * D], F32, tag="pj", name=f"pv{rt}")
                for k in range(6):
                    nc.tensor.matmul(
                        pv[:], xT_sb[:, k, rt * 128:(rt + 1) * 128],
                        wqkv_sb[:, k, 2 * NH * D:3 * NH * D],
                        start=(k == 0), stop=(k == 5))
                nc.vector.tensor_copy(v_sb[:, rt, :, :], pv[:])

        def proj_u(ct, rb):
            """silu(U)^T tile (its 384 u-cols, all rows)."""
            r0, r1 = rb * QB, (rb + 1) * QB
            pu = pjp.tile([128, QB], F32, tag="pj", name=f"pu{ct}{rb}")
            for k in range(6):
                nc.tensor.matmul(pu[:], wu_sb[:, k, ct * 128:(ct + 1) * 128],
                                 xT_sb[:, k, r0:r1],
                                 start=(k == 0), stop=(k == 5))
            usig = wrk.tile([128, QB], BF16, tag="usig")
            nc.scalar.activation(usig[:], pu[:], AF.Sigmoid)
            nc.vector.tensor_mul(ut_sb[:, ct, r0:r1], usig[:], pu[:])

        def attn(qb, p):
            """Windowed causal sigmoid attention for (query block, pair)."""
            nkc = 4 * qb + 4
            q0 = qb * QB
            av = avp.tile([128, QB], F32, tag="av")
            # chunk groups of <=8: scores then A@V per group, so the at-pool
            # (9 bufs) can never wedge the scalar queue against the PE queue
            for g0 in range(0, nkc, 8):
                gkc = range(g0, min(g0 + 8, nkc))
                ats = {}
                for kc in gkc:
                    l0 = max(0, 128 * (kc - 4 * qb))
                    sc = scp.tile([128, 1024], F32, tag="sc")
                    at = atp.tile([128, 1024], BF16, tag="at")
                    for h in range(2):
                        b0 = 64 * h
                        nc.tensor.matmul(
                            sc[:, 512 * h + l0:512 * (h + 1)],
                            kt_sb[b0:b0 + 64, p, kc * 128:(kc + 1) * 128],
                            qt_sb[b0:b0 + 64, p, q0 + l0:q0 + QB],
                            start=True, stop=True)
                    if l0 == 0:
                        nc.scalar.activation(at[:], sc[:], AF.Sigmoid,
                                             scale=0.125)
                    else:
                        for h in range(2):
                            nc.scalar.activation(
                                at[:, 512 * h + l0:512 * (h + 1)],
                                sc[:, 512 * h + l0:512 * (h + 1)],
                                AF.Sigmoid, scale=0.125)
                    if kc >= 4 * qb:
                        for h in range(2):
                            nc.gpsimd.tensor_mul(
                                at[:, 512 * h + l0:512 * h + l0 + 128],
                                at[:, 512 * h + l0:512 * h + l0 + 128],
                                maskb_sb[:])
                    ats[kc] = at
                for kc in gkc:
                    l0 = max(0, 128 * (kc - 4 * qb))
                    for h in range(2):
                        b0 = 64 * h
                        nc.tensor.matmul(
                            av[b0:b0 + 64, l0:QB],
                            v_sb[:, kc, 2 * p + h, :],
                            ats[kc][:, 512 * h + l0:512 * (h + 1)],
                            start=(kc == 0), stop=(kc == nkc - 1),
                            skip_group_check=True)
            nc.vector.tensor_copy(ao_sb[:, p, q0:q0 + QB], av[:])

        def stats_ar(qb):
            """Partial LN stats for block qb -> pair AllReduce (4 KB)."""
            q0 = qb * QB
            ssum = pjp.tile([1, QB], F32, tag="pj", name=f"ss{qb}")
            qsum = pjp.tile([1, QB], F32, tag="pj", name=f"qs{qb}")
            for p in range(NPAIR):
                sq = wrk.tile([128, QB], BF16, tag="sq")
                nc.vector.tensor_mul(sq[:], ao_sb[:, p, q0:q0 + QB],
                                     ao_sb[:, p, q0:q0 + QB])
                nc.tensor.matmul(ssum[:], ones_sb[:], ao_sb[:, p, q0:q0 + QB],
                                 start=(p == 0), stop=(p == 2))
                nc.tensor.matmul(qsum[:], ones_sb[:], sq[:],
                                 start=(p == 0), stop=(p == 2))
            part = ep.tile([1, 2 * QB], F32, tag="part")
            nc.vector.tensor_copy(part[:, 0:QB], ssum[:])
            nc.vector.tensor_copy(part[:, QB:], qsum[:])
            ar_in = dram.tile([1, 2 * QB], F32, tag="arin")
            ar_out = dram.tile([1, 2 * QB], F32, tag="arout")
            nc.gpsimd.dma_start(out=ar_in[:], in_=part[:])
            nc.gpsimd.collective_compute(
                "AllReduce", ALU.add, replica_groups=pairs,
                ins=[ar_in.opt()], outs=[ar_out.opt()])
            return ar_out

        def epilogue(qb, ar_out):
            """LN + gate + partial out-proj + ReduceScatter for block qb."""
            q0 = qb * QB
            st = ep.tile([1, 8 * QB], F32, tag="st")
            nc.sync.dma_start(out=st[:, 0:2 * QB], in_=ar_out[:])
            mu, ex2, vpe, y, t = (st[:, (i + 2) * QB:(i + 3) * QB]
                                  for i in range(5))
            nc.vector.tensor_scalar_mul(mu, st[:, 0:QB], 1.0 / HID)
            nc.vector.tensor_scalar(ex2, st[:, QB:2 * QB], 1.0 / HID, LN_EPS,
                                    ALU.mult, ALU.add)
            nc.vector.tensor_mul(vpe, mu, mu)
            nc.vector.tensor_sub(vpe, ex2, vpe)             # var + eps
            # rsqrt via bit trick + 2 Newton steps (all DVE)
            nc.vector.tensor_scalar(y.bitcast(I32), vpe.bitcast(I32),
                                    1, None, ALU.logical_shift_right)
            nc.vector.tensor_scalar(y.bitcast(I32), y.bitcast(I32),
                                    RSQRT_MAGIC, -1, ALU.subtract, ALU.mult)
            for _ in range(2):
                nc.vector.tensor_mul(t, vpe, y)
                nc.vector.tensor_mul(t, t, y)
                nc.vector.tensor_scalar(t, t, -0.5, 1.5, ALU.mult, ALU.add)
                nc.vector.tensor_mul(y, y, t)
            stb = st.bitcast(BF16)                       # [1, 16*QB] bf16
            mu_b = stb[:, 14 * QB:15 * QB]
            rs_b = stb[:, 15 * QB:16 * QB]
            nc.vector.tensor_copy(mu_b, mu)
            nc.vector.tensor_copy(rs_b, y)
            mu_s = ep.tile([128, QB], BF16, tag="mus")
            rs_s = ep.tile([128, QB], BF16, tag="rss")
            nc.gpsimd.partition_broadcast(mu_s[:], mu_b)
            nc.gpsimd.partition_broadcast(rs_s[:], rs_b)
            gated = gp.tile([128, NPAIR, QB], BF16, tag="gated")
            for p in range(NPAIR):
                d1 = wrk.tile([128, QB], BF16, tag="d1")
                nc.vector.tensor_sub(d1[:], ao_sb[:, p, q0:q0 + QB], mu_s[:])
                nc.vector.tensor_mul(d1[:], d1[:], rs_s[:])
                nc.vector.tensor_mul(gated[:, p, :], d1[:],
                                     ut_sb[:, p, q0:q0 + QB])
            pout = gp.tile([128, CT, QB], BF16, tag="pout")
            for cto in range(CT):
                po = pjp.tile([128, QB], F32, tag="pj", name=f"po{qb}{cto}")
                for k in range(NPAIR):
                    nc.tensor.matmul(
                        po[:], wout_sb[:, k, cto * 128:(cto + 1) * 128],
                        gated[:, k, :], start=(k == 0), stop=(k == 2))
                nc.vector.tensor_copy(pout[:, cto, :], po[:])
            rs_in = dram.tile([CT, 128, QB], BF16, tag="rsin")
            rs_out = dram.tile([NPAIR, 128, QB], BF16, tag="rsout")
            nc.gpsimd.dma_start(out=rs_in.rearrange("c p s -> p c s"),
                                in_=pout[:])
            nc.gpsimd.collective_compute(
                "ReduceScatter", ALU.add, replica_groups=pairs,
                ins=[rs_in.opt()], outs=[rs_out.opt()])
            return rs_out

        def final(qb, rs_out):
            """Residual add + store for block qb (its 384 out-cols)."""
            q0 = qb * QB
            for c in range(NPAIR):
                rsl = wrk.tile([128, QB], BF16, tag="rsl")
                nc.sync.dma_start(out=rsl[:], in_=rs_out[c])
                rt_t = wrk.tile([128, QB], F32, tag="resid")
                nc.sync.dma_start(out=rt_t[:], in_=residT_r[:, c, q0:q0 + QB])
                o_t = wrk.tile([128, QB], F32, tag="osb")
                nc.vector.tensor_add(o_t[:], rsl[:], rt_t[:])
                nc.gpsimd.dma_start(out=out_r[:, c, q0:q0 + QB], in_=o_t[:])

        # ---------- schedule ------------------------------------------
        proj_qkt(0)
        proj_v(0)
        for p in range(NPAIR):          # qb0: keys 0:512 all local
            attn(0, p)
        ar0 = stats_ar(0)
        for sb in (1, 2, 3):
            proj_qkt(sb)
            proj_v(sb)
        for i in range(12):             # U/silu filler, off the att path
            proj_u(i % NPAIR, i // NPAIR)
        for p in range(NPAIR):
            attn(2, p)
        ar2 = stats_ar(2)
        rs0 = epilogue(0, ar0)
        for p in range(NPAIR):
            attn(3, p)
        ar3 = stats_ar(3)
        final(0, rs0)
        rs2 = epilogue(2, ar2)
        for p in range(NPAIR):
            attn(1, p)
        ar1 = stats_ar(1)
        final(2, rs2)
        rs3 = epilogue(3, ar3)
        final(3, rs3)
        rs1 = epilogue(1, ar1)
        final(1, rs1)


# ---------------------------------------------------------------------------
# host side
# ---------------------------------------------------------------------------

def prep_inputs(x, attn_mask, W_proj, b_proj, ln_gamma, ln_beta, W_out, b_out):
    x = np.asarray(x, dtype=np.float32)
    W_proj = np.asarray(W_proj, dtype=np.float32)
    b_proj = np.asarray(b_proj, dtype=np.float32)
    ln_gamma = np.asarray(ln_gamma, dtype=np.float32)
    ln_beta = np.asarray(ln_beta, dtype=np.float32)
    W_out = np.asarray(W_out, dtype=np.float32)
    b_out = np.asarray(b_out, dtype=np.float32)

    tril = np.tril(np.ones((S, S), dtype=bool))
    am = np.asarray(attn_mask)
    if not all(np.array_equal(am[b], tril) for b in range(am.shape[0])):
        raise ValueError("kernel specialized for causal attn_mask")
    if np.any(b_proj != 0) or np.any(ln_beta != 0):
        raise ValueError("kernel specialized for zero b_proj / ln_beta")

    bf = ml_dtypes.bfloat16
    cos, sin = _rope_tables()                      # [S, 64]
    sinN = sin.copy()
    sinN[:, 0:32] = -sinN[:, 0:32]
    cosT = np.tile(cos.T, (2, 1)).astype(bf)       # [128, S]
    sinT = np.tile(sinN.T, (2, 1)).astype(bf)

    iw = np.arange(128)[None, :]
    ii = np.arange(128)[:, None]
    maskb = (iw >= ii).astype(bf)                  # query col >= key row
    ones_k = np.ones((128, 1), dtype=bf)

    Wg = (ln_gamma[:, None] * W_out).astype(np.float32)   # gamma folded
    U_c, V_c, Q_c, K_c = 0, HID, 2 * HID, 3 * HID

    in_maps = []
    for c in range(N_CORES):
        b, hh = c // 2, c % 2
        heads = range(NH * hh, NH * hh + NH)
        hcols = np.concatenate(
            [np.arange(h * D, (h + 1) * D) for h in heads])  # its hidden dims
        w_qkv = np.concatenate(
            [W_proj[:, Q_c + hcols], W_proj[:, K_c + hcols],
             W_proj[:, V_c + hcols]], axis=1).astype(bf)
        ocols = slice(hh * 384, (hh + 1) * 384)    # its out-col half
        xTb = x[b].T                               # [768, 2048]
        residT = (xTb[ocols, :] + b_out[ocols, None]).astype(np.float32)
        in_maps.append(dict(
            xT=np.ascontiguousarray(xTb).astype(bf),
            w_qkv=np.ascontiguousarray(w_qkv),
            w_u=np.ascontiguousarray(W_proj[:, U_c + hcols]).astype(bf),
            w_out=np.ascontiguousarray(Wg[hcols, :]).astype(bf),
            cosT=cosT, sinT=sinT, maskb=maskb, ones_k=ones_k,
            residT=np.ascontiguousarray(residT),
        ))
    return in_maps


def assemble(results, B=4):
    full = np.empty((B, S, HID), dtype=np.float32)
    for c in range(N_CORES):
        b, hh = c // 2, c % 2
        full[b, :, hh * 384:(hh + 1) * 384] = results[c]["out"].T
    return full


_NC_CACHE = {}


def get_nc(ndev=N_CORES):
    if ndev not in _NC_CACHE:
        pairs = [[i, i + 1] for i in range(0, ndev, 2)]
        _NC_CACHE[ndev] = build_nc(ndev, pairs)
    return _NC_CACHE[ndev]


def kernel(**inputs):
    in_maps = prep_inputs(**inputs)
    nc = get_nc(N_CORES)
    res = bass_utils.run_bass_kernel_spmd(
        nc, in_maps, core_ids=list(range(N_CORES)))
    return assemble(res.results)
